# revision 1
# baseline (speedup 1.0000x reference)
"""Trainium2 8-core Bass kernel for nn_Attention_7112465842253.

Token-sharded attention: 512 tokens/core (cores 0-3 = batch 0, 4-7 = batch 1).
Per core: QKV projection in bf16 (q/k in transposed [chan, tok] layout, v in
natural [tok, chan]), RMSNorm via weighted-sumsq matmul + Ln/Exp rstd folded
into the RoPE tables, RoPE as x*C + (P@x)*S with a PE partition-swap matmul,
AllGather of K/V inside each 4-core batch group, non-causal attention in
scoresT layout (k-tokens on partitions; softmax denominator from a ones
column appended to V; Exp fused into the PSUM->SBUF eviction on ScalarE),
then the output projection. Host does layout prep and reassembly.
"""

import numpy as np

B, N, DIN, DIM, H, HD = 2, 2048, 1024, 1024, 16, 64
NCORE = 8
TOK = 512
EPS = 1e-6
BASE = 10000.0
KC = N // 128        # 16 k-token chunks
NHG = 4              # head groups of 4

_CACHE = {}


def _build_nc(dbg=None, single=False, zero_bias=False):
    import concourse.bass as bass
    import concourse.tile as tile
    from concourse import bacc, mybir
    from contextlib import ExitStack

    BF = mybir.dt.bfloat16
    F32 = mybir.dt.float32
    AF = mybir.ActivationFunctionType

    nc = bacc.Bacc(
        "TRN2", target_bir_lowering=False, debug=False,
        num_devices=(1 if single else NCORE),
    )

    # ---------------- DRAM parameters ----------------
    xT = nc.dram_tensor("xT", [DIN, TOK], BF, kind="ExternalInput")
    wqkv = nc.dram_tensor("wqkv", [DIN, 3 * DIM], BF, kind="ExternalInput")
    bqkv = nc.dram_tensor("bqkv", [1, 3 * DIM], BF, kind="ExternalInput")
    wsum = nc.dram_tensor("wsum", [DIN, 2], BF, kind="ExternalInput")
    swp = nc.dram_tensor("swp", [128, 128], BF, kind="ExternalInput")
    tabs = nc.dram_tensor("tabs", [4, 128, TOK], BF, kind="ExternalInput")
    wout = nc.dram_tensor("wout", [DIM, DIN], BF, kind="ExternalInput")
    out = nc.dram_tensor("out", [DIN, TOK], F32, kind="ExternalOutput")
    dbgt = (nc.dram_tensor("dbg", [DIN, TOK], F32, kind="ExternalOutput")
            if dbg else None)

    # internal DRAM
    agin = nc.dram_tensor("agin", [2048 * TOK], BF)   # khatT flat + v flat
    agout = nc.dram_tensor("agout", [4 * 2048 * TOK], BF)
    sescr = nc.dram_tensor("sescr", [1, 4 * TOK], F32)  # recip bcast scratch
    sescr2 = nc.dram_tensor("sescr2", [1, 4 * TOK], F32)  # sumexp staging

    RG = [[0, 1, 2, 3], [4, 5, 6, 7]]

    with tile.TileContext(nc) as tc, ExitStack() as CTX:
        # ---------------- persistent SBUF ----------------
        pp = CTX.enter_context(tc.tile_pool(name="persist", bufs=1))
        qhat = [pp.tile([128, TOK], BF, tag=f"qhat{c}", name=f"qhat{c}") for c in range(8)]
        ktf = [pp.tile([128, N], BF, tag=f"ktf{c}", name=f"ktf{c}") for c in range(8)]
        vaug = [pp.tile([128, 16 * 65], BF, tag=f"vaug{c}", name=f"vaug{c}") for c in range(KC)]
        attnT = [pp.tile([128, TOK], BF, tag=f"attnT{c}", name=f"attnT{c}") for c in range(8)]

        bias_sb = pp.tile([1, 3 * DIM], BF, tag="bias_sb", name="bias_sb")
        wsum_sb = pp.tile([128, 8, 2], BF, tag="wsum_sb", name="wsum_sb")
        swp_sb = pp.tile([128, 128], BF, tag="swp_sb", name="swp_sb")
        ones_t = pp.tile([1, TOK], BF, tag="ones_t", name="ones_t")
        ones_c = pp.tile([1, 128], BF, tag="ones_c", name="ones_c")
        eps_sb = pp.tile([1, 1], F32, tag="eps_sb", name="eps_sb")

        nc.vector.memset(ones_t[:], 1.0)
        nc.vector.memset(ones_c[:], 1.0)
        nc.vector.memset(eps_sb[:], EPS)

        # ---------------- phase 1: qkv + norm + rope + AG ----------------
        p1 = ExitStack()
        wq_pool = p1.enter_context(tc.tile_pool(name="wq", bufs=4))
        vw_pool = p1.enter_context(tc.tile_pool(name="vw", bufs=1))
        sq_pool = p1.enter_context(tc.tile_pool(name="sq", bufs=4))
        qtsb_pool = p1.enter_context(tc.tile_pool(name="qtsb", bufs=9))
        scr1_pool = p1.enter_context(tc.tile_pool(name="scr1", bufs=4))
        kvloc_pool = p1.enter_context(tc.tile_pool(name="kvloc", bufs=1))
        rstd_pool = p1.enter_context(tc.tile_pool(name="rstd", bufs=4))
        # PSUM budget (8 banks): qkvps 3 + swpp 3 + ssqp 1 + bcp 1
        qkvp = p1.enter_context(tc.tile_pool(name="qkvp", bufs=3, space="PSUM"))
        swpp = p1.enter_context(tc.tile_pool(name="swpp", bufs=3, space="PSUM"))
        ssqp = p1.enter_context(tc.tile_pool(name="ssqp", bufs=1, space="PSUM"))
        bcp = p1.enter_context(tc.tile_pool(name="bcp", bufs=1, space="PSUM"))
        misc1_pool = p1.enter_context(tc.tile_pool(name="misc1", bufs=1))
        xt_sb = misc1_pool.tile([128, 8, TOK], BF, tag="xt_sb", name="xt_sb")
        tab_r = [misc1_pool.tile([128, TOK], BF, tag=f"tabr{j}", name=f"tabr{j}") for j in range(4)]
        tab_f = [misc1_pool.tile([128, TOK], BF, tag=f"tabf{j}", name=f"tabf{j}") for j in range(4)]
        for ci in range(8):
            nc.scalar.dma_start(
                out=xt_sb[:, ci, :],
                in_=xT.ap()[ci * 128:(ci + 1) * 128, :],
            )
        nc.scalar.dma_start(out=bias_sb[:], in_=bqkv.ap())
        nc.scalar.dma_start(
            out=wsum_sb[:], in_=wsum.ap().rearrange("(c p) w -> p c w", p=128)
        )
        nc.scalar.dma_start(out=swp_sb[:], in_=swp.ap())
        for j in range(4):
            nc.scalar.dma_start(out=tab_r[j][:], in_=tabs.ap()[j])

        def qkv_chunk_psum(co):
            """psum[128ch, TOK] = sum_ci W[ci, co].T @ xT[ci] + b[co] (x) ones."""
            ps = qkvp.tile([128, TOK], F32, tag="qkvps", name="qkvps")
            wt = wq_pool.tile([128, 8, 128], BF, tag="wt", name="wt")
            nc.sync.dma_start(
                out=wt[:],
                in_=wqkv.ap()[:, co * 128:(co + 1) * 128].rearrange(
                    "(c p) m -> p c m", p=128
                ),
            )
            for ci in range(8):
                nc.tensor.matmul(
                    ps[:], wt[:, ci, :], xt_sb[:, ci, :],
                    start=(ci == 0), stop=(zero_bias and ci == 7),
                )
            if zero_bias:
                pass
            else:
                nc.tensor.matmul(
                    ps[:], bias_sb[:, co * 128:(co + 1) * 128], ones_t[:],
                    start=False, stop=True,
                )
            return ps

        def norm_rope_tensor(which, dst_tiles):
            """which: 0 -> q, 1 -> k. Writes 8 roped bf16 chunks to dst_tiles."""
            co0 = which * 8
            qt_list = []
            ssq = ssqp.tile([1, TOK], F32, tag="ssq", name="ssq")
            for c in range(8):
                ps = qkv_chunk_psum(co0 + c)
                qt = qtsb_pool.tile([128, TOK], BF, tag="qt", name="qt")
                nc.vector.tensor_copy(qt[:], ps[:])
                sqt = sq_pool.tile([128, TOK], BF, tag="sqt", name="sqt")
                nc.vector.tensor_mul(sqt[:], qt[:], qt[:])
                nc.tensor.matmul(
                    ssq[:], wsum_sb[:, c, which:which + 1], sqt[:],
                    start=(c == 0), stop=(c == 7),
                )
                qt_list.append(qt)
            # rstd = exp(-0.5 * ln(var + eps))
            lnv = rstd_pool.tile([1, TOK], F32, tag="lnv", name="lnv")
            nc.scalar.activation(lnv[:], ssq[:], AF.Ln, bias=eps_sb[:])
            rstd = rstd_pool.tile([1, TOK], BF, tag="rstd", name="rstd")
            nc.scalar.activation(rstd[:], lnv[:], AF.Exp, scale=-0.5)
            bc = bcp.tile([128, TOK], F32, tag="bc", name="bc")
            nc.tensor.matmul(bc[:], ones_c[:], rstd[:], start=True, stop=True)
            for j in range(2):
                nc.vector.tensor_mul(
                    tab_f[2 * which + j][:], tab_r[2 * which + j][:], bc[:]
                )
            # rope: dst = qt * C' + (P @ qt) * S'
            for c in range(8):
                sw = swpp.tile([128, TOK], F32, tag="sw", name="sw")
                nc.tensor.matmul(
                    sw[:], swp_sb[:], qt_list[c][:], start=True, stop=True
                )
                m1 = scr1_pool.tile([128, TOK], BF, tag="m1", name="m1")
                nc.vector.tensor_mul(m1[:], qt_list[c][:], tab_f[2 * which][:])
                m2 = scr1_pool.tile([128, TOK], BF, tag="m2", name="m2")
                nc.vector.tensor_mul(m2[:], sw[:], tab_f[2 * which + 1][:])
                nc.vector.tensor_add(dst_tiles[c][:], m1[:], m2[:])
            return qt_list

        vw = vw_pool.tile([128, 8, 2 * TOK], BF, tag="vw", name="vw")
        for ci in range(8):
            nc.sync.dma_start(
                out=vw[:, ci, :],
                in_=wqkv.ap()[ci * 128:(ci + 1) * 128, 2 * DIM:],
            )
        for t4 in range(4):
            for nh in range(2):
                ps = qkvp.tile([128, TOK], F32, tag="qkvps", name="qkvps")
                for ci in range(8):
                    nc.tensor.matmul(
                        ps[:],
                        xt_sb[:, ci, t4 * 128:(t4 + 1) * 128],
                        vw[:, ci, nh * TOK:(nh + 1) * TOK],
                        start=(ci == 0), stop=(zero_bias and ci == 7),
                    )
                if not zero_bias:
                    nc.tensor.matmul(
                        ps[:], ones_c[:],
                        bias_sb[:, 2 * DIM + nh * TOK: 2 * DIM + (nh + 1) * TOK],
                        start=False, stop=True,
                    )
                vl = kvloc_pool.tile([128, TOK], BF, tag=f"vloc{t4}_{nh}", name=f"vloc{t4}_{nh}")
                nc.vector.tensor_copy(vl[:], ps[:])
                dstap = bass.AP(
                    tensor=agin.ap().tensor,
                    offset=1024 * TOK + t4 * 128 * 1024 + nh * TOK,
                    ap=[[1024, 128], [1, TOK]],
                )
                nc.gpsimd.dma_start(out=dstap, in_=vl[:])

        # ---- k first (gates scores), AG-k; v overlaps AG-k; then AG-v, q.
        khat = [kvloc_pool.tile([128, TOK], BF, tag=f"khat{c}", name=f"khat{c}") for c in range(8)]
        norm_rope_tensor(1, khat)
        for c in range(8):
            dstap = bass.AP(
                tensor=agin.ap().tensor,
                offset=c * 128 * TOK,
                ap=[[TOK, 128], [1, TOK]],
            )
            nc.gpsimd.dma_start(out=dstap, in_=khat[c][:])

        if single:
            for r in range(4):
                nc.gpsimd.dma_start(
                    out=bass.AP(tensor=agout.ap().tensor,
                                offset=r * 2048 * TOK, ap=[[1, 2048 * TOK]]),
                    in_=bass.AP(tensor=agin.ap().tensor, offset=0,
                                ap=[[1, 2048 * TOK]]),
                )
        else:
            nc.gpsimd.collective_compute(
                "AllGather",
                mybir.AluOpType.bypass,
                replica_groups=RG,
                ins=[agin.ap().opt()],
                outs=[agout.ap().opt()],
            )

        qt_dbg = norm_rope_tensor(0, qhat)

        def dump8(tiles, cols=None):
            dmp = ExitStack()
            dp = dmp.enter_context(tc.tile_pool(name="dump", bufs=2))
            for c, t in enumerate(tiles):
                f = dp.tile([t.shape[0], TOK], F32, tag="dmp", name="dmp")
                srcap = t[:, cols] if cols is not None else t[:]
                nc.vector.tensor_copy(f[:], srcap)
                nc.gpsimd.dma_start(
                    out=dbgt.ap()[c * 128:c * 128 + t.shape[0], :], in_=f[:]
                )
            dmp.close()

        if dbg == "qt":
            dump8(qt_dbg)
        if dbg == "qhat":
            dump8(qhat)
        if dbg == "khat":
            dump8(khat)

        # ---- post-AG loads
        RSZ = 2048 * TOK
        for c in range(8):
            for r in range(4):
                srcap = bass.AP(
                    tensor=agout.ap().tensor,
                    offset=r * RSZ + c * 128 * TOK,
                    ap=[[TOK, 128], [1, TOK]],
                )
                nc.sync.dma_start(
                    out=ktf[c][:, r * TOK:(r + 1) * TOK], in_=srcap
                )
        for r in range(4):
            for t4 in range(4):
                vc = r * 4 + t4
                srcap = bass.AP(
                    tensor=agout.ap().tensor,
                    offset=r * RSZ + 1024 * TOK + t4 * 128 * 1024,
                    ap=[[1024, 128], [64, 16], [1, 64]],
                )
                dst = vaug[vc][:].rearrange("p (h c) -> p h c", c=65)
                nc.sync.dma_start(out=dst[:, :, 0:64], in_=srcap)
                nc.vector.memset(dst[:, :, 64:65], 1.0)

        if dbg == "ktf":
            dump8(ktf, cols=slice(0, TOK))
        if dbg == "vaug":
            dump8(vaug[:8], cols=slice(0, TOK))

        p1.close()

        # ---------------- phase 2: attention ----------------
        p2 = ExitStack()
        wo_res_pool = CTX.enter_context(tc.tile_pool(name="wores", bufs=1))
        wo_pool = CTX.enter_context(tc.tile_pool(name="wo", bufs=3))
        # PSUM: scp 2x2 banks + avp 4x1 = 8
        scp = CTX.enter_context(tc.tile_pool(name="scp", bufs=2, space="PSUM"))
        avp = p2.enter_context(tc.tile_pool(name="avp", bufs=4, space="PSUM"))
        expt_pool = p2.enter_context(tc.tile_pool(name="expt", bufs=20))
        nrm_pool = p2.enter_context(tc.tile_pool(name="nrm", bufs=2))
        ascr_pool = p2.enter_context(tc.tile_pool(name="ascr", bufs=8))

        expt = {}
        av_tiles = {}
        NHG2 = 8   # groups of 2 heads

        def emit_av(hg, kc):
            for hh in range(2):
                h = hg * 2 + hh
                if kc == 0:
                    av_tiles[(hg, hh)] = avp.tile(
                        [65, TOK], F32, tag="av", name="av")
                nc.tensor.matmul(
                    av_tiles[(hg, hh)][:],
                    vaug[kc][:, h * 65:(h + 1) * 65],
                    expt[(hg, kc)][:, hh * TOK:(hh + 1) * TOK],
                    start=(kc == 0), stop=(kc == KC - 1),
                )

        def emit_normalize(hg):
            # sumexp rows live at PSUM partition 64; stage at partition 64,
            # reshape via DRAM to [128,16] for a fast DVE reciprocal, then
            # broadcast-load back.
            se = nrm_pool.tile([65, 2 * TOK], F32, tag="se", name="se")
            for hh in range(2):
                nc.vector.tensor_copy(
                    se[64:65, hh * TOK:(hh + 1) * TOK],
                    av_tiles[(hg, hh)][64:65, :],
                )
            nc.gpsimd.dma_start(
                out=bass.AP(tensor=sescr2.ap().tensor, offset=0,
                            ap=[[1, 2 * TOK]]),
                in_=se[64:65, :])
            sew = nrm_pool.tile([128, 8], F32, tag="sew", name="sew")
            nc.scalar.dma_start(
                out=sew[:],
                in_=bass.AP(tensor=sescr2.ap().tensor, offset=0,
                            ap=[[8, 128], [1, 8]]),
            )
            rw = nrm_pool.tile([128, 8], F32, tag="rw", name="rw")
            nc.vector.reciprocal(out=rw[:], in_=sew[:])
            nc.gpsimd.dma_start(
                out=bass.AP(tensor=sescr.ap().tensor, offset=0,
                            ap=[[8, 128], [1, 8]]),
                in_=rw[:],
            )
            rbc = nrm_pool.tile([64, 2 * TOK], BF, tag="rbc", name="rbc")
            bcast_src = bass.AP(
                tensor=sescr.ap().tensor,
                offset=0,
                ap=[[0, 64], [1, 2 * TOK]],
            )
            nc.gpsimd.dma_start(out=rbc[:], in_=bcast_src)  # casting DMA
            for hh in range(2):
                h = hg * 2 + hh
                a = ascr_pool.tile([64, TOK], BF, tag="ascr", name="ascr")
                nc.vector.tensor_mul(
                    a[:], av_tiles[(hg, hh)][0:64, :],
                    rbc[:, hh * TOK:(hh + 1) * TOK],
                )
                nc.gpsimd.dma_start(
                    out=attnT[h // 2][(h % 2) * 64:(h % 2) * 64 + 64, :],
                    in_=a[:],
                )

        outps_tiles = {}
        for hg in range(NHG2 + 1):
            for kc in range(KC):
                if hg >= 1:
                    emit_av(hg - 1, kc)
                if hg < NHG2:
                    sc = scp.tile([128, 2 * TOK], F32, tag="sc", name="sc")
                    for hh in range(2):
                        h = hg * 2 + hh
                        nc.tensor.matmul(
                            sc[:, hh * TOK:(hh + 1) * TOK],
                            ktf[h // 2][(h % 2) * 64:(h % 2) * 64 + 64,
                                        kc * 128:(kc + 1) * 128],
                            qhat[h // 2][(h % 2) * 64:(h % 2) * 64 + 64, :],
                            start=True, stop=True,
                        )
                    e = expt_pool.tile([128, 2 * TOK], BF, tag="expt",
                                       name="expt")
                    nc.scalar.activation(e[:], sc[:], AF.Exp)
                    expt[(hg, kc)] = e
            if hg == 5:
                # prefetch the output-projection weights during attention
                wo_res = wo_res_pool.tile([128, 8, 1024], BF, tag="wores",
                                          name="wores")
                for ci in range(8):
                    nc.sync.dma_start(
                        out=wo_res[:, ci, :],
                        in_=wout.ap()[ci * 128:(ci + 1) * 128, :],
                    )
            if hg >= 1:
                emit_normalize(hg - 1)
            if dbg == "expt0" and hg == 0:
                dump8([expt[(0, kc)] for kc in range(8)], cols=slice(0, TOK))
            if dbg == "av0" and hg == 1:
                dump8([av_tiles[(0, hh)] for hh in range(2)])
        p2.close()

        # ---------------- phase 3: output projection ----------------
        p3 = ExitStack()
        for co in range(8):
            ps = scp.tile([128, TOK], F32, tag="sc", name="outps")
            for ci in range(8):
                nc.tensor.matmul(
                    ps[:], wo_res[:, ci, co * 128:(co + 1) * 128],
                    attnT[ci][:],
                    start=(ci == 0), stop=(ci == 7),
                )
            osb = wo_pool.tile([128, TOK], F32, tag="osb", name="osb")
            nc.vector.tensor_copy(osb[:], ps[:])
            nc.gpsimd.dma_start(out=out.ap()[co * 128:(co + 1) * 128, :], in_=osb[:])
        p3.close()

    nc.compile()
    return nc


def _host_prep(inputs):
    import ml_dtypes

    bf16 = ml_dtypes.bfloat16
    x = np.asarray(inputs["x"], np.float32)
    Wqkv = np.asarray(inputs["Wqkv"], np.float32)
    bqkv = np.asarray(inputs["bqkv"], np.float32)
    qs = np.asarray(inputs["q_scale"], np.float32)
    ks = np.asarray(inputs["k_scale"], np.float32)
    Wout = np.asarray(inputs["Wout"], np.float32)

    p64 = np.concatenate([np.arange(0, 64, 2), np.arange(1, 64, 2)])
    perm = np.concatenate([64 * h + p64 for h in range(H)])

    qsp, ksp = qs[perm], ks[perm]
    Wq = Wqkv[:, :DIM][:, perm] * qsp[None, :]
    Wk = Wqkv[:, DIM:2 * DIM][:, perm] * ksp[None, :]
    Wv = Wqkv[:, 2 * DIM:]
    W = np.concatenate([Wq, Wk, Wv], 1).astype(bf16)
    bq = bqkv[:DIM][perm] * qsp
    bk = bqkv[DIM:2 * DIM][perm] * ksp
    bias = np.concatenate([bq, bk, bqkv[2 * DIM:]])[None, :].astype(bf16)
    wsum = np.stack(
        [1.0 / (DIM * qsp ** 2), 1.0 / (DIM * ksp ** 2)], 1
    ).astype(bf16)

    sw = np.arange(128)
    swap = np.where(sw % 64 < 32, sw + 32, sw - 32)
    P = np.zeros((128, 128), np.float32)
    P[swap, np.arange(128)] = 1.0  # (P.T @ x)[m] = x[swap[m]]
    P = P.astype(bf16)

    inv_freq = 1.0 / (BASE ** (np.arange(0, HD, 2).astype(np.float32) / HD))
    pos = np.maximum(np.arange(N) - 1, 0).astype(np.float32)
    ang = pos[:, None] * inv_freq[None, :]
    cosT, sinT = np.cos(ang).T, np.sin(ang).T           # (32, N)
    C128 = np.tile(cosT, (4, 1))                         # (128, N)
    S128 = np.concatenate([-sinT, sinT, -sinT, sinT], 0)

    in_maps = []
    for core in range(NCORE):
        b, sh = core // 4, core % 4
        t0 = sh * TOK
        xTs = np.ascontiguousarray(x[b, t0:t0 + TOK, :].T).astype(bf16)
        tabs = np.stack([
            C128[:, t0:t0 + TOK] * 0.125,
            S128[:, t0:t0 + TOK] * 0.125,
            C128[:, t0:t0 + TOK],
            S128[:, t0:t0 + TOK],
        ]).astype(bf16)
        in_maps.append({
            "xT": xTs,
            "wqkv": W,
            "bqkv": bias,
            "wsum": wsum,
            "swp": P,
            "tabs": np.ascontiguousarray(tabs),
            "wout": Wout.astype(bf16),
        })
    return in_maps


LAST_EXEC_NS = None


def kernel(**inputs):
    global LAST_EXEC_NS
    import os
    from concourse.bass_utils import run_bass_kernel_spmd

    dbg = os.environ.get("KERNEL_DBG") or None
    zb = bool(np.all(np.asarray(inputs["bqkv"]) == 0))
    key = f"nc{dbg}{zb}"
    if key not in _CACHE:
        _CACHE[key] = _build_nc(dbg, zero_bias=zb)
    nc = _CACHE[key]

    in_maps = _host_prep(inputs)
    trace = bool(int(os.environ.get("KERNEL_TRACE", "0")))
    tmpdir = None
    if trace:
        import tempfile
        import concourse.bass_utils as _bu
        _bu.upload_artifacts = lambda d: d  # keep artifacts local
        tmpdir = tempfile.mkdtemp(prefix="ktrace_")
        print("TRACE DIR:", tmpdir)
    res = run_bass_kernel_spmd(
        nc, in_maps, core_ids=list(range(NCORE)), trace=trace, tmpdir=tmpdir
    )
    LAST_EXEC_NS = res.exec_time_ns
    bout = np.asarray(inputs["bout"], np.float32)
    out = np.empty((B, N, DIN), np.float32)
    for core in range(NCORE):
        b, sh = core // 4, core % 4
        t0 = sh * TOK
        out[b, t0:t0 + TOK, :] = res.results[core]["out"].T
    out += bout[None, None, :]
    return out


def kernel_raw(inputs):
    """Debug helper: run and return the per-core raw [1024, 512] outputs."""
    global LAST_EXEC_NS
    import os
    from concourse.bass_utils import run_bass_kernel_spmd

    dbg = os.environ.get("KERNEL_DBG") or None
    zb = bool(np.all(np.asarray(inputs["bqkv"]) == 0))
    key = f"nc{dbg}{zb}"
    if key not in _CACHE:
        _CACHE[key] = _build_nc(dbg, zero_bias=zb)
    nc = _CACHE[key]
    in_maps = _host_prep(inputs)
    res = run_bass_kernel_spmd(nc, in_maps, core_ids=list(range(NCORE)))
    LAST_EXEC_NS = res.exec_time_ns
    key = "dbg" if dbg else "out"
    return [r[key] for r in res.results]



# revision 3
# speedup vs baseline: 1.1882x; 1.1882x over previous
"""Trainium2 8-core Bass kernel for nn_Attention_7112465842253 — v2.

Token-sharded attention: 512 tokens/core (cores 0-3 = batch 0, 4-7 = batch 1).
Restructured vs v1 for an early, never-stalling Exp stream (the Activation
engine is the phase-2 floor at ~130us):
  - K projection first, AllGather-k immediately; Q projection overlaps the
    gather; scores/exp for head-group 0 start as soon as qhat chunk 0 + the
    gathered K land.
  - V projection + AllGather-v are interleaved INTO the score stream (hg1);
    AV matmuls run from a lag-2 backlog queue drained in the PE slack.
  - Weights host-retiled so every DMA has >=2KB contiguous runs (no 2x
    small-element penalty); all bulk DMA on the SP HWDGE queue in dependency
    order; ACT SEQ carries only activations.
  - Odd heads' AV accumulates at partition offset 63 with an ones-FIRST vaug
    layout so sumexp lands at partition 63 and the v-part at 64..127: every
    normalize op is same-partition (no DMA hop to place attnT rows 64..127).
  - bf16 output; host adds bout and casts.
"""

import numpy as np

B, N, DIN, DIM, H, HD = 2, 2048, 1024, 1024, 16, 64
NCORE = 8
TOK = 512
EPS = 1e-6
BASE = 10000.0
KC = N // 128         # 16 k-token chunks of 128
HG = 8                # head-groups of 2 heads
RND = 4               # gather rounds (cores per replica group)

_CACHE = {}


def _build_nc(dbg=None, single=False, zero_bias=False):
    import concourse.bass as bass
    import concourse.tile as tile
    from concourse import bacc, mybir
    from contextlib import ExitStack

    BF = mybir.dt.bfloat16
    F32 = mybir.dt.float32
    AF = mybir.ActivationFunctionType

    nc = bacc.Bacc(
        "TRN2", target_bir_lowering=False, debug=False,
        num_devices=(1 if single else NCORE),
    )

    # ---------------- DRAM parameters ----------------
    xT = nc.dram_tensor("xT", [DIN, TOK], BF, kind="ExternalInput")
    wq_t = nc.dram_tensor("wq_t", [8, 128, 8, 128], BF, kind="ExternalInput")
    wk_t = nc.dram_tensor("wk_t", [8, 128, 8, 128], BF, kind="ExternalInput")
    wv_t = nc.dram_tensor("wv_t", [128, 8, 1024], BF, kind="ExternalInput")
    wo_t = nc.dram_tensor("wo_t", [128, 8, 1024], BF, kind="ExternalInput")
    bqkv = nc.dram_tensor("bqkv", [1, 3 * DIM], BF, kind="ExternalInput")
    wsum = nc.dram_tensor("wsum", [DIN, 2], BF, kind="ExternalInput")
    swp = nc.dram_tensor("swp", [128, 128], BF, kind="ExternalInput")
    tabs = nc.dram_tensor("tabs", [4, 128, TOK], BF, kind="ExternalInput")
    out = nc.dram_tensor("out", [DIN, TOK], BF, kind="ExternalOutput")
    dbgt = (nc.dram_tensor("dbg", [DIN, TOK], F32, kind="ExternalOutput")
            if dbg else None)

    # internal DRAM
    KSZ = DIN * TOK           # one core's k slab (bf16 elems)
    VSZ = TOK * DIM           # one core's v slab
    agk_in = nc.dram_tensor("agk_in", [KSZ], BF)
    agk_out = nc.dram_tensor("agk_out", [RND * KSZ], BF)
    agv_in = nc.dram_tensor("agv_in", [VSZ], BF)
    agv_out = nc.dram_tensor("agv_out", [RND * VSZ], BF)
    sescr = nc.dram_tensor("sescr", [1, 2 * TOK], F32)  # recip bcast scratch

    RG = [[0, 1, 2, 3], [4, 5, 6, 7]]

    with tile.TileContext(nc) as tc, ExitStack() as CTX:
        # ---------------- persistent SBUF ----------------
        # Big tiles with manual slicing: the pool allocator pads every tile
        # to 2KB/partition, so per-chunk tile lists waste ~half their space.
        pp = CTX.enter_context(tc.tile_pool(name="persist", bufs=1))
        qhat_t = pp.tile([128, 8, TOK], BF, tag="qhat", name="qhat")
        qhat = [qhat_t[:, c, :] for c in range(8)]
        ktf_t = pp.tile([128, 8, N], BF, tag="ktf", name="ktf")
        ktf = [ktf_t[:, c, :] for c in range(8)]
        # vaug[kc]: [128 ktok, 8 head-pairs x 130]; pair g (heads 2g, 2g+1):
        # cols [g*130+0, +64) = even-head v, col +64 = ones, cols [+65, +129)
        # = odd-head v, col +129 = ones.  AV lhsT per head = a contiguous
        # 65-col slice with ones LAST -> sumexp lands at psum partition 64.
        vaug_t = pp.tile([128, KC, (H // 2) * 130], BF, tag="vaug",
                         name="vaug")
        vaug = [vaug_t[:, c, :] for c in range(KC)]
        attnT_t = pp.tile([128, 8, TOK], BF, tag="attnT", name="attnT")
        attnT = [attnT_t[:, c, :] for c in range(8)]
        onesm = pp.tile([1, 640], BF, tag="onesm", name="onesm")
        ones_c = onesm[:, 0:128]
        ones_t = onesm[:, 128:640]
        eps_sb = pp.tile([1, 1], F32, tag="eps_sb", name="eps_sb")

        nc.vector.memset(onesm[:], 1.0)
        nc.vector.memset(eps_sb[:], EPS)
        v2 = vaug_t[:].rearrange("p k (h c) -> p (k h) c", c=65)
        nc.vector.memset(v2[:, :, 64:65], 1.0)   # ones column per head

        # ---------------- projection-phase pools ----------------
        # P: lives through phase B (xt/vw for V projection, staging).
        # subA: phase-A-only scratch, closed before expt opens.
        # expt/nrm/wo go on the RIGHT heap side: they outlive P and pool
        # release is strictly LIFO per side.
        P = ExitStack()
        misc_pool = P.enter_context(tc.tile_pool(name="misc", bufs=1))
        vw_pool = P.enter_context(tc.tile_pool(name="vw", bufs=1))
        kv_pool = P.enter_context(tc.tile_pool(name="kvloc", bufs=1))
        stg_pool = P.enter_context(tc.tile_pool(name="stg", bufs=2))

        xt_sb = misc_pool.tile([128, 8, TOK], BF, tag="xt_sb", name="xt_sb")
        bias_sb = (None if zero_bias else
                   misc_pool.tile([1, 3 * DIM], BF, tag="bias_sb",
                                  name="bias_sb"))
        vw = vw_pool.tile([128, 8, 2 * TOK], BF, tag="vw", name="vw")
        vl_t = kv_pool.tile([128, 8, TOK], BF, tag="vl", name="vl")
        vl = [vl_t[:, c, :] for c in range(8)]

        subA = ExitStack()
        a_pool = subA.enter_context(tc.tile_pool(name="aph", bufs=1))
        wt_pool = subA.enter_context(tc.tile_pool(name="wt", bufs=4))
        rstd_pool = subA.enter_context(tc.tile_pool(name="rstd", bufs=1))

        wsum_sb = a_pool.tile([128, 8, 2], BF, tag="wsum_sb", name="wsum_sb")
        swp_sb = a_pool.tile([128, 128], BF, tag="swp_sb", name="swp_sb")
        tabr_t = a_pool.tile([128, 4, TOK], BF, tag="tabr", name="tabr")
        tab_r = [tabr_t[:, j, :] for j in range(4)]
        tabf_t = a_pool.tile([128, 4, TOK], BF, tag="tabf", name="tabf")
        tab_f = [tabf_t[:, j, :] for j in range(4)]
        # manual slot rotation inside big scratch tiles (deps auto-tracked
        # per AP range)
        qt_t = a_pool.tile([128, 9, TOK], BF, tag="qt_t", name="qt_t")
        khat_t = a_pool.tile([128, 8, TOK], BF, tag="khat", name="khat")
        khat = [khat_t[:, c, :] for c in range(8)]
        sq_t = a_pool.tile([128, 2, TOK], BF, tag="sq_t", name="sq_t")
        sw_t = a_pool.tile([128, 2, TOK], BF, tag="sw_t", name="sw_t")
        m_t = a_pool.tile([128, 4, TOK], BF, tag="m_t", name="m_t")

        # PSUM pools (8 banks): pa 2 + pb 1 + pc 3 early; later scp 4 + avp 4.
        pa = ExitStack()
        pa_pool = pa.enter_context(
            tc.tile_pool(name="pa", bufs=2, space="PSUM", side="right"))
        pbc = ExitStack()
        pb_pool = pbc.enter_context(
            tc.tile_pool(name="pb", bufs=2, space="PSUM"))
        pssq_pool = pbc.enter_context(
            tc.tile_pool(name="pssq", bufs=2, space="PSUM"))
        pbc2_pool = pbc.enter_context(
            tc.tile_pool(name="pbc2", bufs=1, space="PSUM"))

        # -------- prefetch DMAs (SP queue, dependency order) --------
        # First k-weight chunk first so PE can start the moment x lands;
        # everything else ordered by first use.
        wt_tiles = {}

        def wt_load(which, c):
            wsrc = wq_t if which == 0 else wk_t
            wt = wt_pool.tile([128, 8, 128], BF, tag="wt",
                              name=f"wt{which}{c}")
            nc.sync.dma_start(out=wt[:], in_=wsrc.ap()[c])
            wt_tiles[(which, c)] = wt

        wt_load(1, 0)
        for ci in range(8):
            nc.sync.dma_start(out=xt_sb[:, ci, :],
                              in_=xT.ap()[ci * 128:(ci + 1) * 128, :])
        nc.sync.dma_start(
            out=wsum_sb[:], in_=wsum.ap().rearrange("(c p) w -> p c w", p=128))
        wt_load(1, 1)
        wt_load(1, 2)
        nc.sync.dma_start(out=tabr_t[:],
                          in_=tabs.ap().rearrange("a p t -> p a t"))
        wt_load(1, 3)
        nc.sync.dma_start(out=swp_sb[:], in_=swp.ap())
        if not zero_bias:
            nc.sync.dma_start(out=bias_sb[:], in_=bqkv.ap())
        # warm both activation tables while ACT is idle
        warm = rstd_pool.tile([1, 1], F32, tag="warm", name="warm")
        nc.scalar.activation(warm[:], eps_sb[:], AF.Ln)
        nc.scalar.activation(warm[:], warm[:], AF.Exp)

        # ---------------- building blocks ----------------
        ssq = {}

        def proj_chunk(which, c):
            """which: 0=q, 1=k. Computes qt chunk c (bf16), accumulates
            weighted sumsq into ssq[which]. Returns the qt tile."""
            ps = pa_pool.tile([128, TOK], F32, tag="pa", name=f"pa_{which}_{c}")
            if (which, c) not in wt_tiles:
                wt_load(which, c)
            wt = wt_tiles.pop((which, c))
            for ci in range(8):
                nc.tensor.matmul(
                    ps[:], wt[:, ci, :], xt_sb[:, ci, :],
                    start=(ci == 0), stop=(zero_bias and ci == 7))
            if not zero_bias:
                co = which * 8 + c
                nc.tensor.matmul(
                    ps[:], bias_sb[:, co * 128:(co + 1) * 128], ones_t[:],
                    start=False, stop=True)
            # manual qt slots: k chunks 0..7; q chunk 0 -> 8; q chunk c>=1
            # reuses k slot c-1 (k-rope c-1 has consumed it by then).
            slot = c if which == 1 else (8 if c == 0 else c - 1)
            qt = qt_t[:, slot, :]
            nc.scalar.activation(qt, ps[:], AF.Copy)
            sqt = sq_t[:, c % 2, :]
            nc.scalar.activation(sqt, ps[:], AF.Square)
            if c == 0:
                ssq[which] = pssq_pool.tile([1, TOK], F32, tag="ssq",
                                           name=f"ssq{which}")
            nc.tensor.matmul(
                ssq[which][:], wsum_sb[:, c, which:which + 1], sqt,
                start=(c == 0), stop=(c == 7))
            return qt

        def rstd_tabs(which):
            """rstd = exp(-0.5*ln(ssq+eps)); tab_f = tab_r * bcast(rstd)."""
            lnv = rstd_pool.tile([1, TOK], F32, tag="lnv", name="lnv")
            nc.scalar.activation(lnv[:], ssq[which][:], AF.Ln, bias=eps_sb[:])
            rstd = rstd_pool.tile([1, TOK], BF, tag="rstd", name="rstd")
            nc.scalar.activation(rstd[:], lnv[:], AF.Exp, scale=-0.5)
            bc = pbc2_pool.tile([128, TOK], F32, tag="bc", name="bc")
            nc.tensor.matmul(bc[:], ones_c[:], rstd[:], start=True, stop=True)
            for j in range(2):
                nc.vector.tensor_mul(
                    tab_f[2 * which + j][:], tab_r[2 * which + j][:], bc[:])

        mslot = [0]

        def rope_chunk(which, c, qt, dst):
            """dst = qt*C' + (P@qt)*S' (rstd folded into C'/S')."""
            sw = pb_pool.tile([128, TOK], F32, tag="sw", name="sw")
            nc.tensor.matmul(sw[:], swp_sb[:], qt, start=True, stop=True)
            swb = sw_t[:, mslot[0] % 2, :]
            m1 = m_t[:, (2 * mslot[0]) % 4, :]
            m2 = m_t[:, (2 * mslot[0] + 1) % 4, :]
            mslot[0] += 1
            nc.scalar.activation(swb, sw[:], AF.Copy)
            nc.vector.tensor_mul(m1, qt, tab_f[2 * which])
            nc.vector.tensor_mul(m2, swb, tab_f[2 * which + 1])
            nc.vector.tensor_add(dst, m1, m2)

        def v_block(t4, nh):
            """v chunk: tokens t4*128.., channels nh*512.. -> vl tile."""
            ps = pa_pool.tile([128, TOK], F32, tag="pa", name=f"pav{t4}{nh}")
            for ci in range(8):
                nc.tensor.matmul(
                    ps[:], xt_sb[:, ci, t4 * 128:(t4 + 1) * 128],
                    vw[:, ci, nh * TOK:(nh + 1) * TOK],
                    start=(ci == 0), stop=(zero_bias and ci == 7))
            if not zero_bias:
                nc.tensor.matmul(
                    ps[:], ones_c[:],
                    bias_sb[:, 2 * DIM + nh * TOK:2 * DIM + (nh + 1) * TOK],
                    start=False, stop=True)
            t = vl[t4 * 2 + nh]
            nc.vector.tensor_copy(t[:], ps[:])

        # ---------------- phase A: k-proj, q-proj, ropes, AGs ----------
        kt_tiles = [proj_chunk(1, c) for c in range(8)]
        qt_tiles = [proj_chunk(0, 0)]
        rstd_tabs(1)
        # all k-ropes up front: the chain is ACT/DVE paced (~0.8us/chunk), so
        # khat completes ~10us earlier than when interleaved with q-proj mms.
        for c in range(8):
            rope_chunk(1, c, kt_tiles[c], khat[c])
        qt_tiles += [proj_chunk(0, c) for c in range(1, 8)]
        # agk stores on SP, emitted after every weight load so they cannot
        # head-of-line block the q-weight prefetches.
        for c in range(8):
            dstap = bass.AP(tensor=agk_in.ap().tensor, offset=c * 128 * TOK,
                            ap=[[TOK, 128], [1, TOK]])
            nc.gpsimd.dma_start(out=dstap, in_=khat[c][:])

        rstd_tabs(0)
        for c in range(8):
            rope_chunk(0, c, qt_tiles[c], qhat[c])

        # close pb/pc before scores psum opens (bank budget); close subA
        # (phase-A scratch) before expt opens.  The AllGather and ktf loads
        # are emitted AFTER the pool transition: pool boundaries serialize
        # against previously-emitted work, and the gather must not gate the
        # exp stream's first tile.
        pbc.close()
        subA.close()
        expt_pool = CTX.enter_context(
            tc.tile_pool(name="expt", bufs=33, side="right"))
        scp_stack = ExitStack()
        scp = scp_stack.enter_context(
            tc.tile_pool(name="scp", bufs=2, space="PSUM"))

        if single:
            nc.gpsimd.dma_start(
                out=bass.AP(tensor=agk_out.ap().tensor, offset=0,
                            ap=[[KSZ, RND], [1, KSZ]]),
                in_=bass.AP(tensor=agk_in.ap().tensor, offset=0,
                            ap=[[0, RND], [1, KSZ]]))
        else:
            nc.gpsimd.collective_compute(
                "AllGather", mybir.AluOpType.bypass, replica_groups=RG,
                ins=[agk_in.ap().opt()], outs=[agk_out.ap().opt()])

        # ktf loads: one DMA per chunk covering all 4 gather rounds
        # (the collective completes as a single event anyway).
        for c in range(8):
            srcap = bass.AP(
                tensor=agk_out.ap().tensor,
                offset=c * 128 * TOK,
                ap=[[TOK, 128], [KSZ, RND], [1, TOK]])
            nc.sync.dma_start(
                out=ktf[c][:].rearrange("p (r t) -> p r t", r=RND),
                in_=srcap)
            if c in (1, 4):
                half = (c == 4)
                nc.sync.dma_start(
                    out=vw[:, :, half * TOK:(half + 1) * TOK],
                    in_=wv_t.ap()[:, :, half * TOK:(half + 1) * TOK])

        # ---------------- phase B: scores/exp stream + V + AV ----------
        expt = {}
        av_tiles = {}
        av_queue = []           # (hg, kc) AV matmuls not yet emitted
        avp_stack = ExitStack()
        avp = None
        nrm_stack = ExitStack()
        nrm_pool = None
        wo_res = None

        def sc_exp(hg, kc):
            sc = scp.tile([128, 2 * TOK], F32, tag="sc", name="sc")
            for hh in range(2):
                h = hg * 2 + hh
                nc.tensor.matmul(
                    sc[:, hh * TOK:(hh + 1) * TOK],
                    ktf[h // 2][(h % 2) * 64:(h % 2) * 64 + 64,
                                kc * 128:(kc + 1) * 128],
                    qhat[h // 2][(h % 2) * 64:(h % 2) * 64 + 64, :],
                    start=True, stop=True)
            e = expt_pool.tile([128, 2 * TOK], BF, tag="expt", name="expt")
            nc.scalar.activation(e[:], sc[:], AF.Exp)
            expt[(hg, kc)] = e

        def emit_av(hg, kc):
            for hh in range(2):
                h = hg * 2 + hh
                if kc == 0:
                    av_tiles[(hg, hh)] = avp.tile(
                        [128, TOK], F32, tag="av", name=f"av{hg}_{hh}")
                t = av_tiles[(hg, hh)]
                nc.tensor.matmul(
                    t[0:65, :], vaug[kc][:, h * 65:(h + 1) * 65],
                    expt[(hg, kc)][:, hh * TOK:(hh + 1) * TOK],
                    start=(kc == 0), stop=(kc == KC - 1))
            expt.pop((hg, kc))

        def emit_normalize(hg):
            # recip of sumexp rows (psum partitions 64 even / 63 odd), bounce
            # through DRAM for the partition-broadcast, then one same-
            # partition mul per head straight into attnT.
            # Engines require equal operand partition offsets (out may sit
            # higher): recip the sumexp rows in place at partition 64, bounce
            # through DRAM for the partition broadcast (DMA moves partitions
            # freely), land BOTH recip rows at partitions 0..63, then
            # same-offset muls write attnT rows 0..63 / 64..127 directly.
            rn = nrm_pool.tile([65, 2 * TOK], F32, tag="rn", name="rn")
            av_e, av_o = av_tiles[(hg, 0)], av_tiles[(hg, 1)]
            nc.vector.reciprocal(out=rn[64:65, 0:TOK], in_=av_e[64:65, :])
            nc.vector.reciprocal(out=rn[64:65, TOK:2 * TOK],
                                 in_=av_o[64:65, :])
            nc.gpsimd.dma_start(
                out=bass.AP(tensor=sescr.ap().tensor, offset=0,
                            ap=[[1, 2 * TOK]]),
                in_=rn[64:65, :])
            rbc = nrm_pool.tile([64, 2 * TOK], F32, tag="rbc", name="rbc")
            nc.gpsimd.dma_start(
                out=rbc[:, 0:TOK],
                in_=bass.AP(tensor=sescr.ap().tensor, offset=0,
                            ap=[[0, 64], [1, TOK]]))
            nc.gpsimd.dma_start(
                out=rbc[:, TOK:2 * TOK],
                in_=bass.AP(tensor=sescr.ap().tensor, offset=TOK,
                            ap=[[0, 64], [1, TOK]]))
            c = hg
            nc.vector.tensor_mul(attnT[c][0:64, :], av_e[0:64, :],
                                 rbc[:, 0:TOK])
            nc.vector.tensor_mul(attnT[c][64:128, :], av_o[0:64, :],
                                 rbc[:, TOK:2 * TOK])

        def drain_av(nmax):
            n = 0
            while av_queue and n < nmax:
                hg_, kc_ = av_queue.pop(0)
                emit_av(hg_, kc_)
                if kc_ == KC - 1:
                    emit_normalize(hg_)
                n += 1

        # vaug fills: staged contiguous loads + strided DVE copies.
        def vaug_fill_from(vc, src_ap):
            """src: [128 tok, 1024 chan] contiguous view; head-pair g occupies
            src cols [g*128, g*128+128): even head first 64, odd next 64."""
            dst = vaug[vc][:].rearrange("p (g c) -> p g c", c=130)
            src3 = src_ap.rearrange("p (g c) -> p g c", c=128)
            nc.vector.tensor_copy(dst[:, :, 0:64], src3[:, :, 0:64])
            nc.vector.tensor_copy(dst[:, :, 65:129], src3[:, :, 64:128])

        def vaug_round(r):
            for t4 in range(4):
                vc = r * 4 + t4
                stg = stg_pool.tile([128, DIM], BF, tag="stg", name="stg")
                nc.sync.dma_start(
                    out=stg[:],
                    in_=bass.AP(tensor=agv_out.ap().tensor,
                                offset=r * VSZ + t4 * 128 * DIM,
                                ap=[[DIM, 128], [1, DIM]]))
                vaug_fill_from(vc, stg[:])

        # hg0 scores; first three V blocks ride in the PE slack
        for kc in range(KC):
            sc_exp(0, kc)
            av_queue.append((0, kc))
            if kc == 8:
                v_block(0, 0)
            elif kc == 11:
                v_block(0, 1)
            elif kc == 14:
                v_block(1, 0)

        # hg1: finish V early, kick AllGather-v, fill vaug per round,
        # start draining hg0's AV backlog once vaug lands.
        for kc in range(KC):
            if kc < 5:
                v_block(1 + (kc + 1) // 2, (kc + 1) % 2)
            sc_exp(1, kc)
            av_queue.append((1, kc))
            if kc == 4:
                pa.close()
                avp = avp_stack.enter_context(
                    tc.tile_pool(name="avp", bufs=4, space="PSUM",
                                 side="right"))
                nrm_pool = nrm_stack.enter_context(
                    tc.tile_pool(name="nrm", bufs=1, side="right"))
            elif kc == 5:
                for t4 in range(4):
                    for nh in range(2):
                        dstap = bass.AP(
                            tensor=agv_in.ap().tensor,
                            offset=t4 * 128 * DIM + nh * TOK,
                            ap=[[DIM, 128], [1, TOK]])
                        nc.gpsimd.dma_start(out=dstap,
                                            in_=vl[t4 * 2 + nh][:])
            elif kc == 6:
                if single:
                    nc.gpsimd.dma_start(
                        out=bass.AP(tensor=agv_out.ap().tensor, offset=0,
                                    ap=[[VSZ, RND], [1, VSZ]]),
                        in_=bass.AP(tensor=agv_in.ap().tensor, offset=0,
                                    ap=[[0, RND], [1, VSZ]]))
                else:
                    nc.gpsimd.collective_compute(
                        "AllGather", mybir.AluOpType.bypass,
                        replica_groups=RG,
                        ins=[agv_in.ap().opt()], outs=[agv_out.ap().opt()])
            elif kc in (7, 9, 11, 13):
                vaug_round((kc - 7) // 2)
            elif kc in (10, 12, 14):
                drain_av(4)

        # ---------------- phase C: steady hg2..7 + AV drain ------------
        P.close()
        wo_pool = nrm_stack.enter_context(
            tc.tile_pool(name="wo", bufs=1, side="right"))

        for hg in range(2, HG):
            for kc in range(KC):
                sc_exp(hg, kc)
                drain_av(2 if kc % 4 == 3 else 1)
                av_queue.append((hg, kc))
            if hg == 5:
                wo_res = wo_pool.tile([128, 8, 1024], BF, tag="wores",
                                      name="wores")
                nc.gpsimd.dma_start(out=wo_res[:], in_=wo_t.ap())
        scp_stack.close()
        drain_av(10 ** 9)

        if dbg:
            dmp_stack = ExitStack()
            dp = dmp_stack.enter_context(tc.tile_pool(name="dump", bufs=2))

            def dump8(tiles, cols=None):
                for c, t in enumerate(tiles):
                    f = dp.tile([128, TOK], F32, tag="dmp", name="dmp")
                    srcap = t[:, cols] if cols is not None else t[:]
                    nc.vector.tensor_copy(f[:], srcap)
                    nc.gpsimd.dma_start(
                        out=dbgt.ap()[c * 128:(c + 1) * 128, :], in_=f[:])

            if dbg == "qhat":
                dump8(qhat)
            elif dbg == "ktf":
                dump8(ktf, cols=slice(0, TOK))
            elif dbg == "vaug":
                dump8(vaug[:8], cols=slice(0, TOK))
            elif dbg == "attnT":
                dump8(attnT)
            dmp_stack.close()

        # ---------------- phase D: output projection ----------------
        op_stack = ExitStack()
        op = op_stack.enter_context(tc.tile_pool(name="op", bufs=4,
                                                 space="PSUM", side="right"))
        ob_pool = op_stack.enter_context(tc.tile_pool(name="ob", bufs=4))
        for co in range(8):
            ps = op.tile([128, TOK], F32, tag="op", name=f"op{co}")
            for ci in range(8):
                nc.tensor.matmul(
                    ps[:], wo_res[:, ci, co * 128:(co + 1) * 128],
                    attnT[ci][:], start=(ci == 0), stop=(ci == 7))
            osb = ob_pool.tile([128, TOK], BF, tag="osb", name="osb")
            nc.vector.tensor_copy(osb[:], ps[:])
            nc.gpsimd.dma_start(out=out.ap()[co * 128:(co + 1) * 128, :],
                                in_=osb[:])
        op_stack.close()
        avp_stack.close()
        nrm_stack.close()

    nc.compile()
    return nc


def _host_prep(inputs):
    import ml_dtypes

    bf16 = ml_dtypes.bfloat16
    x = np.asarray(inputs["x"], np.float32)
    Wqkv = np.asarray(inputs["Wqkv"], np.float32)
    bqkv = np.asarray(inputs["bqkv"], np.float32)
    qs = np.asarray(inputs["q_scale"], np.float32)
    ks = np.asarray(inputs["k_scale"], np.float32)
    Wout = np.asarray(inputs["Wout"], np.float32)

    p64 = np.concatenate([np.arange(0, 64, 2), np.arange(1, 64, 2)])
    perm = np.concatenate([64 * h + p64 for h in range(H)])

    qsp, ksp = qs[perm], ks[perm]
    Wq = Wqkv[:, :DIM][:, perm] * qsp[None, :]
    Wk = Wqkv[:, DIM:2 * DIM][:, perm] * ksp[None, :]
    Wv = Wqkv[:, 2 * DIM:]

    # retile for full-rate DMA: [co, p, ci, m] with 2KB contiguous rows
    wq_t = np.ascontiguousarray(
        Wq.reshape(8, 128, 8, 128).transpose(2, 1, 0, 3)).astype(bf16)
    wk_t = np.ascontiguousarray(
        Wk.reshape(8, 128, 8, 128).transpose(2, 1, 0, 3)).astype(bf16)
    wv_t = np.ascontiguousarray(
        Wv.reshape(8, 128, 1024).transpose(1, 0, 2)).astype(bf16)
    wo_t = np.ascontiguousarray(
        Wout.reshape(8, 128, 1024).transpose(1, 0, 2)).astype(bf16)

    bq = bqkv[:DIM][perm] * qsp
    bk = bqkv[DIM:2 * DIM][perm] * ksp
    bias = np.concatenate([bq, bk, bqkv[2 * DIM:]])[None, :].astype(bf16)
    wsum = np.stack(
        [1.0 / (DIM * qsp ** 2), 1.0 / (DIM * ksp ** 2)], 1).astype(bf16)

    sw = np.arange(128)
    swap = np.where(sw % 64 < 32, sw + 32, sw - 32)
    P = np.zeros((128, 128), np.float32)
    P[swap, np.arange(128)] = 1.0
    P = P.astype(bf16)

    inv_freq = 1.0 / (BASE ** (np.arange(0, HD, 2).astype(np.float32) / HD))
    pos = np.maximum(np.arange(N) - 1, 0).astype(np.float32)
    ang = pos[:, None] * inv_freq[None, :]
    cosT, sinT = np.cos(ang).T, np.sin(ang).T
    C128 = np.tile(cosT, (4, 1))
    S128 = np.concatenate([-sinT, sinT, -sinT, sinT], 0)

    in_maps = []
    for core in range(NCORE):
        b, sh = core // 4, core % 4
        t0 = sh * TOK
        xTs = np.ascontiguousarray(x[b, t0:t0 + TOK, :].T).astype(bf16)
        tabs = np.stack([
            C128[:, t0:t0 + TOK] * 0.125,
            S128[:, t0:t0 + TOK] * 0.125,
            C128[:, t0:t0 + TOK],
            S128[:, t0:t0 + TOK],
        ]).astype(bf16)
        in_maps.append({
            "xT": xTs,
            "wq_t": wq_t, "wk_t": wk_t, "wv_t": wv_t, "wo_t": wo_t,
            "bqkv": bias,
            "wsum": wsum,
            "swp": P,
            "tabs": np.ascontiguousarray(tabs),
        })
    return in_maps


LAST_EXEC_NS = None


def kernel(**inputs):
    global LAST_EXEC_NS
    import os
    from concourse.bass_utils import run_bass_kernel_spmd

    dbg = os.environ.get("KERNEL_DBG") or None
    zb = bool(np.all(np.asarray(inputs["bqkv"]) == 0))
    key = f"nc{dbg}{zb}"
    if key not in _CACHE:
        _CACHE[key] = _build_nc(dbg, zero_bias=zb)
    nc = _CACHE[key]

    in_maps = _host_prep(inputs)
    res = run_bass_kernel_spmd(nc, in_maps, core_ids=list(range(NCORE)))
    LAST_EXEC_NS = res.exec_time_ns
    bout = np.asarray(inputs["bout"], np.float32)
    out = np.empty((B, N, DIN), np.float32)
    for core in range(NCORE):
        b, sh = core // 4, core % 4
        t0 = sh * TOK
        out[b, t0:t0 + TOK, :] = np.asarray(
            res.results[core]["out"], np.float32).T
    out += bout[None, None, :]
    return out


def kernel_raw(inputs):
    """Debug helper: run and return per-core raw outputs (dbg or out)."""
    global LAST_EXEC_NS
    import os
    from concourse.bass_utils import run_bass_kernel_spmd

    dbg = os.environ.get("KERNEL_DBG") or None
    zb = bool(np.all(np.asarray(inputs["bqkv"]) == 0))
    key = f"nc{dbg}{zb}"
    if key not in _CACHE:
        _CACHE[key] = _build_nc(dbg, zero_bias=zb)
    nc = _CACHE[key]
    in_maps = _host_prep(inputs)
    res = run_bass_kernel_spmd(nc, in_maps, core_ids=list(range(NCORE)))
    LAST_EXEC_NS = res.exec_time_ns
    key = "dbg" if dbg else "out"
    return [np.asarray(r[key]) for r in res.results]
